# revision 28
# baseline (speedup 1.0000x reference)
"""Paged KV-cache decode attention with ALiBi (Baichuan-style), fused
QKV + attention + output projection, tensor-parallel over heads across
8 Trainium2 NeuronCores.

Final version (~90us, 1.43x over the 129.5us v6 baseline; rel err
9.3e-3 vs the 2e-2 gate). Per core (5 head-slots, slot index =
emission order = descending alibi window so small slots land in the
tail):

  - All projections in natural orientation (lhsT = hT chunk [128(E),4],
    rhs = W chunk [128(E),512/128] streaming): PE runs at the weight
    stream floor (~10.9us per projection) instead of LDWEIGHTS-bound
    ~28us, and the wide streams keep the PE activity monitor at
    2.4GHz. qT/kT recovered via 10 small PE transposes ([4,128] ->
    [128,4], identity shipped in the hT DMA's last 4 columns).
  - wk/wv quantized host-side to fp8 e3m4 (power-of-2 scale, descale
    folded into the PSUM->SBUF evac copy): halves their HBM bytes; the
    k/v-projection error only affects the newly decoded token
    (9.3e-3 total). wq/ow stay fp16 (q/o errors hit every position;
    measured wq-e3m4 adds +4.5e-3 for zero speedup - tail is not
    DMA-bound).
  - ALiBi window truncation at TCUT=9 with balanced head->slot
    permutation (error contribution negligible vs fp8 term).
  - V new-token scatter: the last (newest) chunk of every (slot,seq)
    lives in a separate newV tile [128, B*HPC, 128] packed so seq b's
    new-token row is one CONTIGUOUS 640-element write -> 4 DMAs
    instead of 20 serial ones (which cost ~17us in v8).
  - DMA order (one gpsimd/SWDGE queue): hT+ident, bias, wk, wq, wv,
    Kt(slots), newV, [Vt(i), ow(i)] per slot. Weight groups pool
    (bufs=5) so the stream self-paces against PE consumption; K/V/ow
    tiles are single-use exact-size (no pool-reuse stalls).
  - Attention: all slots' scores+bias+exp first (fills the
    v-proj->scatter->Vt latency window), then per slot: AV chains
    (last chunk from newV), per-slot softmax normalization, and
    slot-streamed o_proj. NOTE: fusing scores into the per-slot loop
    or lagging the norm by one slot both measured SLOWER (cross-engine
    FIFO convoys); this exact interleave is load-bearing.
  - o_proj: all 10 jg groups accumulate slot-by-slot in 5 PSUM banks,
    2 accumulators per bank via col tiling (tile_position (0,0) and
    (0,64)); jg 0-4 evac to partitions 0-3, jg 5-9 to 64-67 of
    out_sb, two final stores. The tail after the last slot is ~4us.
  - Host: fp16/fp8 casts, head-permuted weight/cache packing, additive
    fp32 bias (-1e30 masking), per-core shards; partial outputs summed
    on host (no collective).
"""

import math
import os
import sys
from contextlib import ExitStack

import numpy as np
import ml_dtypes

sys.path.insert(0, "/opt/trn_rl_repo")

B = 4
E = 5120
H = 40
D = 128
BS = 16
NB = 512
MB = 128
S = MB * BS  # 2048
NCORES = 8
HPC = H // NCORES   # 5 head-slots per core
EPC = HPC * D       # 640

NEG = -1.0e30
GK = 10             # E-chunks (of 128) per qkv weight DMA group
NG = 40 // GK
TCUT = 9.0          # alibi bias cutoff (dropped weight <= ~e^-9 rel)

E3 = ml_dtypes.float8_e3m4


def _alibi_slopes(num_heads):
    cp2 = 2 ** int(math.floor(math.log2(num_heads)))
    base = 2.0 ** (-(2.0 ** (-(math.log2(cp2) - 3))))
    slopes = base ** np.arange(1, cp2 + 1, dtype=np.float64)
    if cp2 != num_heads:
        extra_base = 2.0 ** (-(2.0 ** (-(math.log2(2 * cp2) - 3))))
        n_rem = min(cp2, num_heads - cp2)
        extra = extra_base ** np.arange(1, 1 + 2 * n_rem, 2, dtype=np.float64)
        slopes = np.concatenate([slopes, extra])
    return slopes.astype(np.float32)


def _head_partition(pos, nch):
    """Rank heads by alibi window; slot i (in emission order, biggest
    window first) gets ranks [gi*8, (gi+1)*8) where gi runs over groups
    sorted by descending window. Returns (head_ranks, m) with
    head_ranks[i] = the 8 global head ids of slot i (per-core pick is
    head_ranks[i][core]) and m[i][b] = kept trailing chunks."""
    win = np.ceil(TCUT / _alibi_slopes(H).astype(np.float64)).astype(np.int64)
    order = np.argsort(win, kind="stable")
    groups = []
    for g in range(HPC):
        ids = order[g * NCORES:(g + 1) * NCORES]
        wmax = int(win[ids].max())
        groups.append((wmax, [int(x) for x in ids]))
    groups.sort(key=lambda t: -t[0])
    head_ranks = tuple(tuple(ids) for _, ids in groups)
    m = tuple(
        tuple(nch[b] - max(0, (pos[b] - wmax) // 128) for b in range(B))
        for wmax, _ in groups
    )
    return head_ranks, m


_PROGRAM_CACHE = {}
LAST_RESULTS = None  # BassKernelResults of the most recent run (for test.py)


def _build_program(pos, nch, m, rs_q, rs_k, rs_v):
    """Build the SPMD Bass program. pos/nch/m and the fp8 descale
    factors are baked statically (same for all cores)."""
    import concourse.bacc as bacc
    import concourse.bass as bass
    import concourse.tile as tile
    from concourse import mybir

    f32 = mybir.dt.float32
    f16 = mybir.dt.float16
    f8 = mybir.dt.float8e3
    nc = bacc.Bacc()

    # Vt holds chunks 0..m-2 per (i,b); the last chunk lives in newV.
    mv = [[m[i][b] - 1 for b in range(B)] for i in range(HPC)]
    vcnt = [sum(mv[i]) for i in range(HPC)]
    kcnt = [sum(m[i]) for i in range(HPC)]
    ksoff = [0]
    vsoff = [0]
    for i in range(HPC):
        ksoff.append(ksoff[-1] + kcnt[i])
        vsoff.append(vsoff[-1] + vcnt[i])
    KCH, VCH = ksoff[-1], vsoff[-1]
    kmoff = [[sum(m[i][:b]) for b in range(B)] for i in range(HPC)]
    vmoff = [[sum(mv[i][:b]) for b in range(B)] for i in range(HPC)]
    c0 = [[nch[b] - m[i][b] for b in range(B)] for i in range(HPC)]

    hT = nc.declare_dram_parameter("hT", [128, 40 * B + B], f16, isOutput=False)
    wq_d = nc.declare_dram_parameter("wq", [128, 40, EPC], f16, isOutput=False)
    wk_d = nc.declare_dram_parameter("wk", [128, 40, EPC], f8, isOutput=False)
    wv_d = nc.declare_dram_parameter("wv", [128, 40, EPC], f8, isOutput=False)
    ow_d = nc.declare_dram_parameter("ow", [HPC, 128, E], f16, isOutput=False)
    kt_d = nc.declare_dram_parameter("kt", [128, KCH * 128], f16, isOutput=False)
    vt_d = nc.declare_dram_parameter("vt", [128, max(VCH, 1), D], f16, isOutput=False)
    nv_d = nc.declare_dram_parameter("nv", [128, B * HPC, D], f16, isOutput=False)
    bias_d = nc.declare_dram_parameter("bias", [128, B * HPC * 16], f32, isOutput=False)
    outT = nc.declare_dram_parameter("outT", [B, E], f32, isOutput=True)

    with tile.TileContext(nc) as tc, ExitStack() as ctx:
        consts = ctx.enter_context(tc.tile_pool(name="consts", bufs=1))
        wpool = ctx.enter_context(tc.tile_pool(name="wpool", bufs=6))
        psA = ctx.enter_context(tc.tile_pool(name="psA", bufs=3, space="PSUM"))
        psO = ctx.enter_context(tc.tile_pool(name="psO", bufs=5, space="PSUM"))

        hT_sb = consts.tile([128, 40 * B + B], f16)
        bias_sb = consts.tile([128, B * HPC * 16], f32)
        # 4x4 identity for PE transposes rides in the last hT columns
        ident = hT_sb[0:B, 40 * B:40 * B + B]
        ones_col = consts.tile([128, 1], f16)
        nc.vector.memset(ones_col[:], 1.0)
        ones_row = consts.tile([1, 128], f16)
        nc.vector.memset(ones_row[:], 1.0)

        qT_sb = consts.tile([128, HPC * B], f16)    # [d, i*B+b]
        kT_sb = consts.tile([128, HPC * B], f16)
        colsum_sb = consts.tile([128, HPC * B], f32)
        aoT_sb = consts.tile([128, HPC * B], f32)
        attn_nT = consts.tile([128, HPC * B], f16)
        q_nat = consts.tile([B, EPC], f16)
        k_nat = consts.tile([B, EPC], f16)
        v_nat = consts.tile([B, EPC], f16)
        # o_proj output: jg 0-4 on partitions 0-3, jg 5-9 on 64-67
        out_sb = consts.tile([128, E // 2], f32)

        Kts = [consts.tile([128, kcnt[i] * 128], f16, name=f"K{i}") for i in range(HPC)]
        Vts = [consts.tile([128, max(vcnt[i], 1), D], f16, name=f"V{i}")
               for i in range(HPC)]
        newV = consts.tile([128, B * HPC, D], f16)
        ow_ts = [consts.tile([128, E], f16, name=f"ow{i}") for i in range(HPC)]

        # ---- bulk DMA stream on the gpsimd (SWDGE) queue, in order ----
        nc.gpsimd.dma_start(out=hT_sb[:], in_=hT[:])
        nc.gpsimd.dma_start(out=bias_sb[:], in_=bias_d[:])
        wq_t, wk_t, wv_t = [], [], []

        def wgroup(dram, lst, nm, dt):
            t = wpool.tile([128, GK, EPC], dt, tag="w", name=f"{nm}{len(lst)}")
            nc.gpsimd.dma_start(out=t[:], in_=dram[:, len(lst) * GK:(len(lst) + 1) * GK, :])
            lst.append(t)

        for g in range(NG):
            wgroup(wk_d, wk_t, "wk", f8)
        for g in range(NG):
            wgroup(wq_d, wq_t, "wq", f16)
        for g in range(NG):
            wgroup(wv_d, wv_t, "wv", f8)
        for i in range(HPC):
            nc.gpsimd.dma_start(
                out=Kts[i][:], in_=kt_d[:, ksoff[i] * 128:ksoff[i + 1] * 128]
            )
        nc.gpsimd.dma_start(out=newV[:], in_=nv_d[:])
        for i in range(HPC):
            if vcnt[i]:
                nc.gpsimd.dma_start(
                    out=Vts[i][:], in_=vt_d[:, vsoff[i]:vsoff[i + 1], :]
                )
            nc.gpsimd.dma_start(out=ow_ts[i][:], in_=ow_d[i, :, :])

        # ---- projections (natural orientation) ----
        def proj(lst, nat, scale, pool=None, tag="ps"):
            pool = pool or psA
            p0 = pool.tile([B, 512], f32, tag=tag)
            p1 = pool.tile([B, EPC - 512], f32, tag=tag)
            for g in range(NG):
                wt = lst[g]
                for kl in range(GK):
                    kc = g * GK + kl
                    nc.tensor.matmul(
                        p0[:], lhsT=hT_sb[:, kc * B:(kc + 1) * B], rhs=wt[:, kl, :512],
                        start=(kc == 0), stop=(kc == 39),
                    )
                    nc.tensor.matmul(
                        p1[:], lhsT=hT_sb[:, kc * B:(kc + 1) * B], rhs=wt[:, kl, 512:],
                        start=(kc == 0), stop=(kc == 39),
                    )
            if scale == 1.0:
                nc.scalar.copy(nat[:, :512], p0[:])
                nc.scalar.copy(nat[:, 512:], p1[:])
            else:
                nc.scalar.mul(nat[:, :512], p0[:], scale)
                nc.scalar.mul(nat[:, 512:], p1[:], scale)

        def transpose_to(nat, dst_sb, i):
            tp = psA.tile([128, B], f16, tag="ps", name=f"tp{i}")
            nc.tensor.transpose(tp[:], nat[:, i * 128:(i + 1) * 128], ident[:])
            nc.scalar.copy(dst_sb[:, i * B:(i + 1) * B], tp[:])

        proj(wk_t, k_nat, rs_k)
        for i in range(HPC):
            transpose_to(k_nat, kT_sb, i)
        proj(wq_t, q_nat, rs_q)  # q pre-scaled by 1/sqrt(D) host-side
        for i in range(HPC):
            transpose_to(q_nat, qT_sb, i)
        proj(wv_t, v_nat, rs_v, pool=psO, tag="po")

        # V new-token scatter: one contiguous DMA per sequence
        for b in range(B):
            p = pos[b]
            nc.sync.dma_start(
                out=newV[p % 128:p % 128 + 1, b * HPC:(b + 1) * HPC, :],
                in_=v_nat[b:b + 1, :],
            )

        # o_proj accumulators: 10 jg groups packed 2-per-bank (col tiling:
        # jg<5 at partitions 0-3, jg>=5 at partitions 64-67 of bank jg%5)
        opsO_t = [psO.tile([128, 512], f32, tag="po", name=f"po{t}") for t in range(5)]

        def o_acc(jg):
            t = opsO_t[jg % 5]
            return (t[0:B, :], (0, 0)) if jg < 5 else (t[64:64 + B, :], (0, 64))


        # ---- per-slot attention, software-pipelined depth 2:
        # scores(0), scores(1), then per slot [AV(i), norm(i), o_proj(i),
        # scores(i+2)] so the next slots' score matmuls fill the
        # cross-engine round-trip gaps of slot i's chain.
        attn = {}

        def emit_scores(i):
            Kt = Kts[i]
            # K new-token scatter (same partitions: DVE copy)
            for b in range(B):
                lp = (kmoff[i][b] + pos[b] // 128 - c0[i][b]) * 128 + pos[b] % 128
                nc.vector.tensor_copy(
                    Kt[:, lp:lp + 1], kT_sb[:, i * B + b:i * B + b + 1]
                )
            for b in range(B):
                n = m[i][b]
                col = i * B + b
                sc_ps = psA.tile([128, 16], f32, tag="ps", name=f"sc_{i}_{b}")
                for c in range(n):
                    nc.tensor.matmul(
                        sc_ps[:, c:c + 1],
                        lhsT=Kt[:, (kmoff[i][b] + c) * 128:(kmoff[i][b] + c + 1) * 128],
                        rhs=qT_sb[:, col:col + 1],
                        start=True, stop=True,
                    )
                s_sb = consts.tile([128, 16], f32, name=f"s_{i}_{b}")
                nc.vector.tensor_add(
                    s_sb[:, :n], sc_ps[:, :n],
                    bias_sb[:, (b * HPC + i) * 16:(b * HPC + i) * 16 + n],
                )
                a_sb = consts.tile([128, 16], f16, name=f"at_{i}_{b}")
                nc.scalar.activation(
                    a_sb[:, :n], s_sb[:, :n],
                    func=mybir.ActivationFunctionType.Exp,
                    accum_out=colsum_sb[:, col:col + 1],
                )
                attn[(i, b)] = a_sb

        for i in range(HPC):
            emit_scores(i)
        recips = []

        def norm_oproj(i):
            rb_ps = psA.tile([128, B], f32, tag="ps", name=f"rb{i}")
            nc.tensor.matmul(
                rb_ps[:], lhsT=ones_row[:], rhs=recips[i][:], start=True, stop=True,
            )
            rb_sb = consts.tile([128, B], f32, name=f"rbs{i}")
            nc.vector.tensor_copy(rb_sb[:], rb_ps[:])
            nc.vector.tensor_mul(
                attn_nT[:, i * B:(i + 1) * B], aoT_sb[:, i * B:(i + 1) * B], rb_sb[:]
            )
            for jg in range(E // 512):
                acc, tpos = o_acc(jg)
                nc.tensor.matmul(
                    acc,
                    lhsT=attn_nT[:, i * B:(i + 1) * B],
                    rhs=ow_ts[i][:, jg * 512:(jg + 1) * 512],
                    start=(i == 0), stop=(i == HPC - 1),
                    tile_position=tpos,
                )

        for i in range(HPC):
            Vt = Vts[i]
            for b in range(B):
                n = m[i][b]
                col = i * B + b
                ao_ps = psA.tile([128, 1], f32, tag="ps", name=f"ao_{i}_{b}")
                for c in range(n - 1):
                    nc.tensor.matmul(
                        ao_ps[:],
                        lhsT=Vt[:, vmoff[i][b] + c, :],
                        rhs=attn[(i, b)][:, c:c + 1],
                        start=(c == 0), stop=False,
                    )
                nc.tensor.matmul(
                    ao_ps[:],
                    lhsT=newV[:, b * HPC + i, :],
                    rhs=attn[(i, b)][:, n - 1:n],
                    start=(n == 1), stop=True,
                )
                nc.scalar.copy(aoT_sb[:, col:col + 1], ao_ps[:])
            # per-slot normalization + slot-streamed o_proj (fp16 copy of
            # the per-partition partials so the sum matmul is 1-pass;
            # partials are <= ~2.5e3, well inside fp16 range)
            cs16 = consts.tile([128, B], f16, name=f"cs{i}")
            nc.vector.tensor_copy(cs16[:], colsum_sb[:, i * B:(i + 1) * B])
            sums_ps = psA.tile([1, B], f32, tag="ps", name=f"sum{i}")
            nc.tensor.matmul(
                sums_ps[:], lhsT=ones_col[:], rhs=cs16[:], start=True, stop=True,
            )
            recip_sb = consts.tile([1, B], f32, name=f"rc{i}")
            nc.vector.reciprocal(recip_sb[:], sums_ps[:])
            # fp16 copy so the broadcast matmul is 1-pass, not 4-pass fp32
            recip16 = consts.tile([1, B], f16, name=f"rch{i}")
            nc.vector.tensor_copy(recip16[:], recip_sb[:])
            recips.append(recip16)
            norm_oproj(i)

        # ---- o_proj evac + stores ----
        for jg in range(E // 512):
            acc, _ = o_acc(jg)
            dst = out_sb[0:B, (jg % 5) * 512:(jg % 5 + 1) * 512] if jg < 5 else \
                out_sb[64:64 + B, (jg % 5) * 512:(jg % 5 + 1) * 512]
            if jg % 2 == 0:
                nc.scalar.copy(dst, acc)
            else:
                nc.vector.tensor_copy(dst, acc)

        nc.sync.dma_start(out=outT[:, :E // 2], in_=out_sb[0:B, :])
        nc.sync.dma_start(out=outT[:, E // 2:], in_=out_sb[64:64 + B, :])

    nc.compile()
    return nc


def _pow2_scale(x, cap):
    mx = float(np.abs(x).max())
    return 2.0 ** math.floor(math.log2(cap / mx))


def _prepare_core_inputs(core, hidden16, wq16, wk8, wv8, o16, k16, v16, bt, sl,
                         pos, nch, head_ranks, m):
    """Per-core staged arrays; slot index = emission order."""
    heads = [head_ranks[i][core] for i in range(HPC)]

    def wlayout(Wh):  # Wh: [E, EPC] -> [128, 40, EPC]
        return np.ascontiguousarray(Wh.reshape(40, 128, EPC).transpose(1, 0, 2))

    wq = wlayout(wq16.reshape(E, H, D)[:, heads, :].reshape(E, EPC))
    wk = wlayout(wk8.reshape(E, H, D)[:, heads, :].reshape(E, EPC))
    wv = wlayout(wv8.reshape(E, H, D)[:, heads, :].reshape(E, EPC))

    mv = [[m[i][b] - 1 for b in range(B)] for i in range(HPC)]
    kcnt = [sum(m[i]) for i in range(HPC)]
    vcnt = [sum(mv[i]) for i in range(HPC)]
    ksoff = [0]
    vsoff = [0]
    for i in range(HPC):
        ksoff.append(ksoff[-1] + kcnt[i])
        vsoff.append(vsoff[-1] + vcnt[i])
    KCH, VCH = ksoff[-1], vsoff[-1]
    kmoff = [[sum(m[i][:b]) for b in range(B)] for i in range(HPC)]
    vmoff = [[sum(mv[i][:b]) for b in range(B)] for i in range(HPC)]
    c0 = [[nch[b] - m[i][b] for b in range(B)] for i in range(HPC)]

    kg = k16[:, heads]  # [NB, HPC, BS, D]
    vg = v16[:, heads]
    kt = np.zeros((D, KCH * 128), np.float16)
    vt = np.zeros((128, max(VCH, 1), D), np.float16)
    nv = np.zeros((128, B * HPC, D), np.float16)
    for b in range(B):
        sd = nch[b] * 128
        blocks = bt[b][: sd // BS]
        kk = kg[blocks].transpose(1, 0, 2, 3).reshape(HPC, sd, D)
        vv = vg[blocks].transpose(1, 0, 2, 3).reshape(HPC, sd, D)
        for i in range(HPC):
            base = ksoff[i] + kmoff[i][b]
            n = m[i][b]
            ksl = kk[i, c0[i][b] * 128: sd]              # [n*128, D]
            kt[:, base * 128:(base + n) * 128] = ksl.T
            vsl = vv[i, c0[i][b] * 128: sd].reshape(n, 128, D)
            vb = vsoff[i] + vmoff[i][b]
            vt[:, vb:vb + n - 1, :] = vsl[:-1].transpose(1, 0, 2)
            nv[:, b * HPC + i, :] = vsl[-1]

    slopes = _alibi_slopes(H)[heads]
    t_in = np.arange(128)[:, None]
    biasa = np.full((128, B, HPC, 16), NEG, np.float32)
    for b in range(B):
        for i in range(HPC):
            n = m[i][b]
            tg = ((c0[i][b] + np.arange(n))[None, :] * 128 + t_in).astype(np.float32)
            val = slopes[i] * (tg - np.float32(pos[b]))
            val[tg >= sl[b]] = NEG
            biasa[:, b, i, :n] = val

    hTf = np.zeros((128, 40 * B + B), np.float16)
    hTf[:, :40 * B] = hidden16.T.reshape(40, 128, B).transpose(1, 0, 2).reshape(
        128, 40 * B)
    hTf[:B, 40 * B:] = np.eye(B, dtype=np.float16)
    owr = np.ascontiguousarray(o16.reshape(H, D, E)[heads])

    return dict(
        hT=hTf, wq=wq, wk=wk, wv=wv, ow=owr, kt=kt, vt=vt, nv=nv,
        bias=np.ascontiguousarray(biasa.reshape(128, B * HPC * 16)),
    )


def kernel(**inputs):
    global LAST_RESULTS
    hidden = np.asarray(inputs["hidden_states"], np.float32)
    qkv_w = np.asarray(inputs["qkv_weight"], np.float32)
    o_w = np.asarray(inputs["o_proj_weight"], np.float32)
    k_cache = np.asarray(inputs["k_cache"], np.float32)
    v_cache = np.asarray(inputs["v_cache"], np.float32)
    bt = np.asarray(inputs["block_tables"]).astype(np.int64)
    sl = np.asarray(inputs["sequence_lengths"]).astype(np.int64)

    pos = tuple(int(x) - 1 for x in sl)
    nch = tuple(int(math.ceil(int(x) / 128)) for x in sl)
    head_ranks, m = _head_partition(pos, nch)

    hidden16 = hidden.astype(np.float16)
    s_q = 1.0
    wq16 = (qkv_w[0] * np.float32(D ** -0.5)).astype(np.float16)
    s_k = _pow2_scale(qkv_w[1], 14.0)
    s_v = _pow2_scale(qkv_w[2], 14.0)
    wk8 = (qkv_w[1] * np.float32(s_k)).astype(E3)
    wv8 = (qkv_w[2] * np.float32(s_v)).astype(E3)
    o16 = o_w.astype(np.float16)
    k16 = k_cache.astype(np.float16)
    v16 = v_cache.astype(np.float16)

    in_maps = [
        _prepare_core_inputs(c, hidden16, wq16, wk8, wv8, o16, k16, v16, bt, sl,
                             pos, nch, head_ranks, m)
        for c in range(NCORES)
    ]

    key = (pos, nch, m, s_q, s_k, s_v)
    if key not in _PROGRAM_CACHE:
        _PROGRAM_CACHE[key] = _build_program(pos, nch, m, 1.0 / s_q, 1.0 / s_k, 1.0 / s_v)
    nc = _PROGRAM_CACHE[key]

    from concourse.bass_utils import run_bass_kernel_spmd

    res = run_bass_kernel_spmd(
        nc,
        in_maps,
        core_ids=list(range(NCORES)),
        trace=bool(os.environ.get("BASS_TRACE")),
    )
    LAST_RESULTS = res

    out = np.zeros((B, E), np.float64)
    for c in range(NCORES):
        out += np.asarray(res.results[c]["outT"]).astype(np.float64)
    return out.astype(np.float32)


# revision 29
# speedup vs baseline: 1.0193x; 1.0193x over previous
"""Paged KV-cache decode attention with ALiBi (Baichuan-style), fused
QKV + attention + output projection, tensor-parallel over heads across
8 Trainium2 NeuronCores.

Final version (~90us, 1.43x over the 129.5us v6 baseline; rel err
9.3e-3 vs the 2e-2 gate). Per core (5 head-slots, slot index =
emission order = descending alibi window so small slots land in the
tail):

  - All projections in natural orientation (lhsT = hT chunk [128(E),4],
    rhs = W chunk [128(E),512/128] streaming): PE runs at the weight
    stream floor (~10.9us per projection) instead of LDWEIGHTS-bound
    ~28us, and the wide streams keep the PE activity monitor at
    2.4GHz. qT/kT recovered via 10 small PE transposes ([4,128] ->
    [128,4], identity shipped in the hT DMA's last 4 columns).
  - wk/wv quantized host-side to fp8 e3m4 (power-of-2 scale, descale
    folded into the PSUM->SBUF evac copy): halves their HBM bytes; the
    k/v-projection error only affects the newly decoded token
    (9.3e-3 total). wq/ow stay fp16 (q/o errors hit every position;
    measured wq-e3m4 adds +4.5e-3 for zero speedup - tail is not
    DMA-bound).
  - ALiBi window truncation at TCUT=9 with balanced head->slot
    permutation (error contribution negligible vs fp8 term).
  - V new-token scatter: the last (newest) chunk of every (slot,seq)
    lives in a separate newV tile [128, B*HPC, 128] packed so seq b's
    new-token row is one CONTIGUOUS 640-element write -> 4 DMAs
    instead of 20 serial ones (which cost ~17us in v8).
  - DMA order (one gpsimd/SWDGE queue): hT+ident, bias, wk, wq, wv,
    Kt(slots), newV, [Vt(i), ow(i)] per slot. Weight groups pool
    (bufs=5) so the stream self-paces against PE consumption; K/V/ow
    tiles are single-use exact-size (no pool-reuse stalls).
  - Attention: all slots' scores+bias+exp first (fills the
    v-proj->scatter->Vt latency window), then per slot: AV chains
    (last chunk from newV), per-slot softmax normalization, and
    slot-streamed o_proj. NOTE: fusing scores into the per-slot loop
    or lagging the norm by one slot both measured SLOWER (cross-engine
    FIFO convoys); this exact interleave is load-bearing.
  - o_proj: all 10 jg groups accumulate slot-by-slot in 5 PSUM banks,
    2 accumulators per bank via col tiling (tile_position (0,0) and
    (0,64)); jg 0-4 evac to partitions 0-3, jg 5-9 to 64-67 of
    out_sb, two final stores. The tail after the last slot is ~4us.
  - Host: fp16/fp8 casts, head-permuted weight/cache packing, additive
    fp32 bias (-1e30 masking), per-core shards; partial outputs summed
    on host (no collective).
"""

import math
import os
import sys
from contextlib import ExitStack

import numpy as np
import ml_dtypes

sys.path.insert(0, "/opt/trn_rl_repo")

B = 4
E = 5120
H = 40
D = 128
BS = 16
NB = 512
MB = 128
S = MB * BS  # 2048
NCORES = 8
HPC = H // NCORES   # 5 head-slots per core
EPC = HPC * D       # 640

NEG = -1.0e30
GK = 10             # E-chunks (of 128) per qkv weight DMA group
NG = 40 // GK
TCUT = 9.0          # alibi bias cutoff (dropped weight <= ~e^-9 rel)

E3 = ml_dtypes.float8_e3m4


def _alibi_slopes(num_heads):
    cp2 = 2 ** int(math.floor(math.log2(num_heads)))
    base = 2.0 ** (-(2.0 ** (-(math.log2(cp2) - 3))))
    slopes = base ** np.arange(1, cp2 + 1, dtype=np.float64)
    if cp2 != num_heads:
        extra_base = 2.0 ** (-(2.0 ** (-(math.log2(2 * cp2) - 3))))
        n_rem = min(cp2, num_heads - cp2)
        extra = extra_base ** np.arange(1, 1 + 2 * n_rem, 2, dtype=np.float64)
        slopes = np.concatenate([slopes, extra])
    return slopes.astype(np.float32)


def _head_partition(pos, nch):
    """Rank heads by alibi window; slot i (in emission order, biggest
    window first) gets ranks [gi*8, (gi+1)*8) where gi runs over groups
    sorted by descending window. Returns (head_ranks, m) with
    head_ranks[i] = the 8 global head ids of slot i (per-core pick is
    head_ranks[i][core]) and m[i][b] = kept trailing chunks."""
    win = np.ceil(TCUT / _alibi_slopes(H).astype(np.float64)).astype(np.int64)
    order = np.argsort(win, kind="stable")
    groups = []
    for g in range(HPC):
        ids = order[g * NCORES:(g + 1) * NCORES]
        wmax = int(win[ids].max())
        groups.append((wmax, [int(x) for x in ids]))
    groups.sort(key=lambda t: -t[0])
    head_ranks = tuple(tuple(ids) for _, ids in groups)
    m = tuple(
        tuple(nch[b] - max(0, (pos[b] - wmax) // 128) for b in range(B))
        for wmax, _ in groups
    )
    return head_ranks, m


_PROGRAM_CACHE = {}
LAST_RESULTS = None  # BassKernelResults of the most recent run (for test.py)


def _build_program(pos, nch, m, rs_q, rs_k, rs_v):
    """Build the SPMD Bass program. pos/nch/m and the fp8 descale
    factors are baked statically (same for all cores)."""
    import concourse.bacc as bacc
    import concourse.bass as bass
    import concourse.tile as tile
    from concourse import mybir

    f32 = mybir.dt.float32
    f16 = mybir.dt.float16
    f8 = mybir.dt.float8e3
    nc = bacc.Bacc()

    # Vt holds chunks 0..m-2 per (i,b); the last chunk lives in newV.
    mv = [[m[i][b] - 1 for b in range(B)] for i in range(HPC)]
    vcnt = [sum(mv[i]) for i in range(HPC)]
    kcnt = [sum(m[i]) for i in range(HPC)]
    ksoff = [0]
    vsoff = [0]
    for i in range(HPC):
        ksoff.append(ksoff[-1] + kcnt[i])
        vsoff.append(vsoff[-1] + vcnt[i])
    KCH, VCH = ksoff[-1], vsoff[-1]
    kmoff = [[sum(m[i][:b]) for b in range(B)] for i in range(HPC)]
    vmoff = [[sum(mv[i][:b]) for b in range(B)] for i in range(HPC)]
    c0 = [[nch[b] - m[i][b] for b in range(B)] for i in range(HPC)]

    hT = nc.declare_dram_parameter("hT", [128, 40 * B + B], f16, isOutput=False)
    wq_d = nc.declare_dram_parameter("wq", [128, 40, EPC], f16, isOutput=False)
    wk_d = nc.declare_dram_parameter("wk", [128, 40, EPC], f8, isOutput=False)
    wv_d = nc.declare_dram_parameter("wv", [128, 40, EPC], f8, isOutput=False)
    ow_d = nc.declare_dram_parameter("ow", [HPC, 128, E], f16, isOutput=False)
    kt_d = nc.declare_dram_parameter("kt", [128, KCH * 128], f16, isOutput=False)
    vt_d = nc.declare_dram_parameter("vt", [128, max(VCH, 1), D], f16, isOutput=False)
    nv_d = nc.declare_dram_parameter("nv", [128, B * HPC, D], f16, isOutput=False)
    bias_d = nc.declare_dram_parameter("bias", [128, B * HPC * 16], f32, isOutput=False)
    outT = nc.declare_dram_parameter("outT", [B, E], f32, isOutput=True)

    with tile.TileContext(nc) as tc, ExitStack() as ctx:
        consts = ctx.enter_context(tc.tile_pool(name="consts", bufs=1))
        wpool = ctx.enter_context(tc.tile_pool(name="wpool", bufs=6))
        psA = ctx.enter_context(tc.tile_pool(name="psA", bufs=3, space="PSUM"))
        psO = ctx.enter_context(tc.tile_pool(name="psO", bufs=5, space="PSUM"))

        hT_sb = consts.tile([128, 40 * B + B], f16)
        bias_sb = consts.tile([128, B * HPC * 16], f32)
        # 4x4 identity for PE transposes rides in the last hT columns
        ident = hT_sb[0:B, 40 * B:40 * B + B]
        ones_col = consts.tile([128, 1], f32)
        nc.vector.memset(ones_col[:], 1.0)
        ones_row = consts.tile([1, 128], f16)
        nc.vector.memset(ones_row[:], 1.0)

        qT_sb = consts.tile([128, HPC * B], f16)    # [d, i*B+b]
        kT_sb = consts.tile([128, HPC * B], f16)
        colsum_sb = consts.tile([128, HPC * B], f32)
        aoT_sb = consts.tile([128, HPC * B], f32)
        attn_nT = consts.tile([128, HPC * B], f16)
        q_nat = consts.tile([B, EPC], f16)
        k_nat = consts.tile([B, EPC], f16)
        v_nat = consts.tile([B, EPC], f16)
        # o_proj output: jg 0-4 on partitions 0-3, jg 5-9 on 64-67
        out_sb = consts.tile([128, E // 2], f32)

        Kts = [consts.tile([128, kcnt[i] * 128], f16, name=f"K{i}") for i in range(HPC)]
        Vts = [consts.tile([128, max(vcnt[i], 1), D], f16, name=f"V{i}")
               for i in range(HPC)]
        newV = consts.tile([128, B * HPC, D], f16)
        ow_ts = [consts.tile([128, E], f16, name=f"ow{i}") for i in range(HPC)]

        # ---- bulk DMA stream on the gpsimd (SWDGE) queue, in order ----
        nc.gpsimd.dma_start(out=hT_sb[:], in_=hT[:])
        nc.gpsimd.dma_start(out=bias_sb[:], in_=bias_d[:])
        wq_t, wk_t, wv_t = [], [], []

        def wgroup(dram, lst, nm, dt):
            t = wpool.tile([128, GK, EPC], dt, tag="w", name=f"{nm}{len(lst)}")
            nc.gpsimd.dma_start(out=t[:], in_=dram[:, len(lst) * GK:(len(lst) + 1) * GK, :])
            lst.append(t)

        for g in range(NG):
            wgroup(wk_d, wk_t, "wk", f8)
        for g in range(NG):
            wgroup(wq_d, wq_t, "wq", f16)
        for g in range(NG):
            wgroup(wv_d, wv_t, "wv", f8)
        for i in range(HPC):
            nc.gpsimd.dma_start(
                out=Kts[i][:], in_=kt_d[:, ksoff[i] * 128:ksoff[i + 1] * 128]
            )
        nc.gpsimd.dma_start(out=newV[:], in_=nv_d[:])
        for i in range(HPC):
            if vcnt[i]:
                nc.gpsimd.dma_start(
                    out=Vts[i][:], in_=vt_d[:, vsoff[i]:vsoff[i + 1], :]
                )
            nc.gpsimd.dma_start(out=ow_ts[i][:], in_=ow_d[i, :, :])

        # ---- projections (natural orientation) ----
        def proj(lst, nat, scale, pool=None, tag="ps"):
            pool = pool or psA
            p0 = pool.tile([B, 512], f32, tag=tag)
            p1 = pool.tile([B, EPC - 512], f32, tag=tag)
            for g in range(NG):
                wt = lst[g]
                for kl in range(GK):
                    kc = g * GK + kl
                    nc.tensor.matmul(
                        p0[:], lhsT=hT_sb[:, kc * B:(kc + 1) * B], rhs=wt[:, kl, :512],
                        start=(kc == 0), stop=(kc == 39),
                    )
                    nc.tensor.matmul(
                        p1[:], lhsT=hT_sb[:, kc * B:(kc + 1) * B], rhs=wt[:, kl, 512:],
                        start=(kc == 0), stop=(kc == 39),
                    )
            if scale == 1.0:
                nc.scalar.copy(nat[:, :512], p0[:])
                nc.scalar.copy(nat[:, 512:], p1[:])
            else:
                nc.scalar.mul(nat[:, :512], p0[:], scale)
                nc.scalar.mul(nat[:, 512:], p1[:], scale)

        def transpose_to(nat, dst_sb, i):
            tp = psA.tile([128, B], f16, tag="ps", name=f"tp{i}")
            nc.tensor.transpose(tp[:], nat[:, i * 128:(i + 1) * 128], ident[:])
            nc.scalar.copy(dst_sb[:, i * B:(i + 1) * B], tp[:])

        proj(wk_t, k_nat, rs_k)
        for i in range(HPC):
            transpose_to(k_nat, kT_sb, i)
        proj(wq_t, q_nat, rs_q)  # q pre-scaled by 1/sqrt(D) host-side
        for i in range(HPC):
            transpose_to(q_nat, qT_sb, i)
        proj(wv_t, v_nat, rs_v, pool=psO, tag="po")

        # V new-token scatter: one contiguous DMA per sequence
        for b in range(B):
            p = pos[b]
            nc.sync.dma_start(
                out=newV[p % 128:p % 128 + 1, b * HPC:(b + 1) * HPC, :],
                in_=v_nat[b:b + 1, :],
            )

        # o_proj accumulators: 10 jg groups packed 2-per-bank (col tiling:
        # jg<5 at partitions 0-3, jg>=5 at partitions 64-67 of bank jg%5)
        opsO_t = [psO.tile([128, 512], f32, tag="po", name=f"po{t}") for t in range(5)]

        def o_acc(jg):
            t = opsO_t[jg % 5]
            return (t[0:B, :], (0, 0)) if jg < 5 else (t[64:64 + B, :], (0, 64))


        # ---- per-slot attention, software-pipelined depth 2:
        # scores(0), scores(1), then per slot [AV(i), norm(i), o_proj(i),
        # scores(i+2)] so the next slots' score matmuls fill the
        # cross-engine round-trip gaps of slot i's chain.
        attn = {}

        def emit_scores(i):
            Kt = Kts[i]
            # K new-token scatter (same partitions: DVE copy)
            for b in range(B):
                lp = (kmoff[i][b] + pos[b] // 128 - c0[i][b]) * 128 + pos[b] % 128
                nc.vector.tensor_copy(
                    Kt[:, lp:lp + 1], kT_sb[:, i * B + b:i * B + b + 1]
                )
            for b in range(B):
                n = m[i][b]
                col = i * B + b
                sc_ps = psA.tile([128, 16], f32, tag="ps", name=f"sc_{i}_{b}")
                for c in range(n):
                    nc.tensor.matmul(
                        sc_ps[:, c:c + 1],
                        lhsT=Kt[:, (kmoff[i][b] + c) * 128:(kmoff[i][b] + c + 1) * 128],
                        rhs=qT_sb[:, col:col + 1],
                        start=True, stop=True,
                    )
                s_sb = consts.tile([128, 16], f32, name=f"s_{i}_{b}")
                nc.vector.tensor_add(
                    s_sb[:, :n], sc_ps[:, :n],
                    bias_sb[:, (b * HPC + i) * 16:(b * HPC + i) * 16 + n],
                )
                a_sb = consts.tile([128, 16], f16, name=f"at_{i}_{b}")
                nc.scalar.activation(
                    a_sb[:, :n], s_sb[:, :n],
                    func=mybir.ActivationFunctionType.Exp,
                    accum_out=colsum_sb[:, col:col + 1],
                )
                attn[(i, b)] = a_sb

        for i in range(HPC):
            emit_scores(i)
        recips = []

        def norm_oproj(i):
            rb_ps = psA.tile([128, B], f32, tag="ps", name=f"rb{i}")
            nc.tensor.matmul(
                rb_ps[:], lhsT=ones_row[:], rhs=recips[i][:], start=True, stop=True,
            )
            rb_sb = consts.tile([128, B], f32, name=f"rbs{i}")
            nc.vector.tensor_copy(rb_sb[:], rb_ps[:])
            nc.vector.tensor_mul(
                attn_nT[:, i * B:(i + 1) * B], aoT_sb[:, i * B:(i + 1) * B], rb_sb[:]
            )
            for jg in range(E // 512):
                acc, tpos = o_acc(jg)
                nc.tensor.matmul(
                    acc,
                    lhsT=attn_nT[:, i * B:(i + 1) * B],
                    rhs=ow_ts[i][:, jg * 512:(jg + 1) * 512],
                    start=(i == 0), stop=(i == HPC - 1),
                    tile_position=tpos,
                )

        for i in range(HPC):
            Vt = Vts[i]
            for b in range(B):
                n = m[i][b]
                col = i * B + b
                ao_ps = psA.tile([128, 1], f32, tag="ps", name=f"ao_{i}_{b}")
                for c in range(n - 1):
                    nc.tensor.matmul(
                        ao_ps[:],
                        lhsT=Vt[:, vmoff[i][b] + c, :],
                        rhs=attn[(i, b)][:, c:c + 1],
                        start=(c == 0), stop=False,
                    )
                nc.tensor.matmul(
                    ao_ps[:],
                    lhsT=newV[:, b * HPC + i, :],
                    rhs=attn[(i, b)][:, n - 1:n],
                    start=(n == 1), stop=True,
                )
                nc.scalar.copy(aoT_sb[:, col:col + 1], ao_ps[:])
            # per-slot normalization + slot-streamed o_proj
            sums_ps = psA.tile([1, B], f32, tag="ps", name=f"sum{i}")
            nc.tensor.matmul(
                sums_ps[:], lhsT=ones_col[:],
                rhs=colsum_sb[:, i * B:(i + 1) * B], start=True, stop=True,
            )
            recip_sb = consts.tile([1, B], f32, name=f"rc{i}")
            nc.vector.reciprocal(recip_sb[:], sums_ps[:])
            # fp16 copy so the broadcast matmul is 1-pass, not 4-pass fp32
            recip16 = consts.tile([1, B], f16, name=f"rch{i}")
            nc.vector.tensor_copy(recip16[:], recip_sb[:])
            recips.append(recip16)
            norm_oproj(i)

        # ---- o_proj evac + stores ----
        for jg in range(E // 512):
            acc, _ = o_acc(jg)
            dst = out_sb[0:B, (jg % 5) * 512:(jg % 5 + 1) * 512] if jg < 5 else \
                out_sb[64:64 + B, (jg % 5) * 512:(jg % 5 + 1) * 512]
            if jg % 2 == 0:
                nc.scalar.copy(dst, acc)
            else:
                nc.vector.tensor_copy(dst, acc)

        nc.sync.dma_start(out=outT[:, :E // 2], in_=out_sb[0:B, :])
        nc.sync.dma_start(out=outT[:, E // 2:], in_=out_sb[64:64 + B, :])

    nc.compile()
    return nc


def _pow2_scale(x, cap):
    mx = float(np.abs(x).max())
    return 2.0 ** math.floor(math.log2(cap / mx))


def _prepare_core_inputs(core, hidden16, wq16, wk8, wv8, o16, k16, v16, bt, sl,
                         pos, nch, head_ranks, m):
    """Per-core staged arrays; slot index = emission order."""
    heads = [head_ranks[i][core] for i in range(HPC)]

    def wlayout(Wh):  # Wh: [E, EPC] -> [128, 40, EPC]
        return np.ascontiguousarray(Wh.reshape(40, 128, EPC).transpose(1, 0, 2))

    wq = wlayout(wq16.reshape(E, H, D)[:, heads, :].reshape(E, EPC))
    wk = wlayout(wk8.reshape(E, H, D)[:, heads, :].reshape(E, EPC))
    wv = wlayout(wv8.reshape(E, H, D)[:, heads, :].reshape(E, EPC))

    mv = [[m[i][b] - 1 for b in range(B)] for i in range(HPC)]
    kcnt = [sum(m[i]) for i in range(HPC)]
    vcnt = [sum(mv[i]) for i in range(HPC)]
    ksoff = [0]
    vsoff = [0]
    for i in range(HPC):
        ksoff.append(ksoff[-1] + kcnt[i])
        vsoff.append(vsoff[-1] + vcnt[i])
    KCH, VCH = ksoff[-1], vsoff[-1]
    kmoff = [[sum(m[i][:b]) for b in range(B)] for i in range(HPC)]
    vmoff = [[sum(mv[i][:b]) for b in range(B)] for i in range(HPC)]
    c0 = [[nch[b] - m[i][b] for b in range(B)] for i in range(HPC)]

    kg = k16[:, heads]  # [NB, HPC, BS, D]
    vg = v16[:, heads]
    kt = np.zeros((D, KCH * 128), np.float16)
    vt = np.zeros((128, max(VCH, 1), D), np.float16)
    nv = np.zeros((128, B * HPC, D), np.float16)
    for b in range(B):
        sd = nch[b] * 128
        blocks = bt[b][: sd // BS]
        kk = kg[blocks].transpose(1, 0, 2, 3).reshape(HPC, sd, D)
        vv = vg[blocks].transpose(1, 0, 2, 3).reshape(HPC, sd, D)
        for i in range(HPC):
            base = ksoff[i] + kmoff[i][b]
            n = m[i][b]
            ksl = kk[i, c0[i][b] * 128: sd]              # [n*128, D]
            kt[:, base * 128:(base + n) * 128] = ksl.T
            vsl = vv[i, c0[i][b] * 128: sd].reshape(n, 128, D)
            vb = vsoff[i] + vmoff[i][b]
            vt[:, vb:vb + n - 1, :] = vsl[:-1].transpose(1, 0, 2)
            nv[:, b * HPC + i, :] = vsl[-1]

    slopes = _alibi_slopes(H)[heads]
    t_in = np.arange(128)[:, None]
    biasa = np.full((128, B, HPC, 16), NEG, np.float32)
    for b in range(B):
        for i in range(HPC):
            n = m[i][b]
            tg = ((c0[i][b] + np.arange(n))[None, :] * 128 + t_in).astype(np.float32)
            val = slopes[i] * (tg - np.float32(pos[b]))
            val[tg >= sl[b]] = NEG
            biasa[:, b, i, :n] = val

    hTf = np.zeros((128, 40 * B + B), np.float16)
    hTf[:, :40 * B] = hidden16.T.reshape(40, 128, B).transpose(1, 0, 2).reshape(
        128, 40 * B)
    hTf[:B, 40 * B:] = np.eye(B, dtype=np.float16)
    owr = np.ascontiguousarray(o16.reshape(H, D, E)[heads])

    return dict(
        hT=hTf, wq=wq, wk=wk, wv=wv, ow=owr, kt=kt, vt=vt, nv=nv,
        bias=np.ascontiguousarray(biasa.reshape(128, B * HPC * 16)),
    )


def kernel(**inputs):
    global LAST_RESULTS
    hidden = np.asarray(inputs["hidden_states"], np.float32)
    qkv_w = np.asarray(inputs["qkv_weight"], np.float32)
    o_w = np.asarray(inputs["o_proj_weight"], np.float32)
    k_cache = np.asarray(inputs["k_cache"], np.float32)
    v_cache = np.asarray(inputs["v_cache"], np.float32)
    bt = np.asarray(inputs["block_tables"]).astype(np.int64)
    sl = np.asarray(inputs["sequence_lengths"]).astype(np.int64)

    pos = tuple(int(x) - 1 for x in sl)
    nch = tuple(int(math.ceil(int(x) / 128)) for x in sl)
    head_ranks, m = _head_partition(pos, nch)

    hidden16 = hidden.astype(np.float16)
    s_q = 1.0
    wq16 = (qkv_w[0] * np.float32(D ** -0.5)).astype(np.float16)
    s_k = _pow2_scale(qkv_w[1], 14.0)
    s_v = _pow2_scale(qkv_w[2], 14.0)
    wk8 = (qkv_w[1] * np.float32(s_k)).astype(E3)
    wv8 = (qkv_w[2] * np.float32(s_v)).astype(E3)
    o16 = o_w.astype(np.float16)
    k16 = k_cache.astype(np.float16)
    v16 = v_cache.astype(np.float16)

    in_maps = [
        _prepare_core_inputs(c, hidden16, wq16, wk8, wv8, o16, k16, v16, bt, sl,
                             pos, nch, head_ranks, m)
        for c in range(NCORES)
    ]

    key = (pos, nch, m, s_q, s_k, s_v)
    if key not in _PROGRAM_CACHE:
        _PROGRAM_CACHE[key] = _build_program(pos, nch, m, 1.0 / s_q, 1.0 / s_k, 1.0 / s_v)
    nc = _PROGRAM_CACHE[key]

    from concourse.bass_utils import run_bass_kernel_spmd

    res = run_bass_kernel_spmd(
        nc,
        in_maps,
        core_ids=list(range(NCORES)),
        trace=bool(os.environ.get("BASS_TRACE")),
    )
    LAST_RESULTS = res

    out = np.zeros((B, E), np.float64)
    for c in range(NCORES):
        out += np.asarray(res.results[c]["outT"]).astype(np.float64)
    return out.astype(np.float32)
